# revision 5
# baseline (speedup 1.0000x reference)
"""Trainium2 Bass kernel for nn_DiffusionHead: 100-step diffusion sampling of a
tiny MLP head (130->128->128->1) over a batch of 262144 rows.

v2 design ("3-set PSUM rotation", ACT-bound ~6.6ms target):
  - Pure data parallel over 8 NeuronCores, 32768 rows/core, batch split in two
    halves of 16384 that alternate at STEP granularity (the x-recurrence tail
    of one half hides under the other half's 8 compute slots).
  - Layout: features d (128) on partitions, batch n on the free dim.
  - Slot = 1024 batch cols. Per slot: z1 = W1a@ctx + w1x@x_row accumulated in
    one rotating PSUM tile [128,1024] (2 banks); ONE 1024-wide Silu (bias =
    b1 + time_emb[t]*W1[129] folded per-partition) -> h1 bf16; z2 = W2@h1 into
    the next rotating PSUM tile; ONE 1024-wide Silu -> h2 bf16; pred = W3c@h2
    (2 x 512-col matmuls, zero-padded stationary trick) into a dedicated
    double-buffered 1-bank pred pool; DVE drains pred [2,512] -> SBUF; DMA
    scatters it into the square pred_sq [128,128] layout.
  - PSUM budget: 3 z-tiles x 2 banks + 2 pred bufs x 1 bank = 8 banks exactly.
  - ACT does exactly 2 x (1024+222)cyc instructions per slot => ~66.4us/step;
    PE does 4096 cyc/slot (~82% duty, stays HAM-warm at 2.4GHz).
  - x-update runs on DVE in the square [128,128] layout with schedule constants
    folded as immediates; x_row (bf16 row layout for the rank-1 matmul) is
    rebuilt via cast + SBUF->SBUF DMA once per half-step.
"""

import os
import numpy as np
import ml_dtypes

import bass_rust
import concourse.bass as bass
import concourse.bacc as bacc
import concourse.mybir as mybir
from concourse import tile
from concourse import bass_utils

DEP_ORDER = bass_rust.DependencyInfo(sync=False, no_sync=True)

B = 262144
D = 128
T_STEPS = 100
N_CORES = 8
NPC = B // N_CORES          # 32768 rows per core
HALF = NPC // 2             # 16384
SLOT = 1024                 # batch cols per slot
NSLOT = HALF // SLOT        # 8 slots per half
SQF = HALF // D             # 128 free cols of the square layout
BETA_START = 1e-4
BETA_END = 0.02

F32 = mybir.dt.float32
BF16 = mybir.dt.bfloat16


def _schedule(n_steps):
    """Compile-time diffusion schedule constants (pure linspace math; no input
    data involved). Computed in float64 for accuracy."""
    betas = np.linspace(BETA_START, BETA_END, T_STEPS, dtype=np.float64)
    alphas = 1.0 - betas
    acp = np.cumprod(alphas)
    a_t = 1.0 / np.sqrt(alphas)                       # x coefficient
    b_t = -betas / (np.sqrt(1.0 - acp) * np.sqrt(alphas))  # pred coefficient
    c_t = np.sqrt(betas)                              # eps coefficient
    return a_t, b_t, c_t


def build(n_steps=T_STEPS, dt=BF16):
    nc = bacc.Bacc("TRN2", target_bir_lowering=False, debug=False)

    # ---------------- DRAM tensors (per-core inputs) ----------------
    ctxT = nc.dram_tensor("ctxT", [D, NPC], dt, kind="ExternalInput").ap()
    noise = nc.dram_tensor("noise", [T_STEPS, NPC], F32, kind="ExternalInput").ap()
    x0 = nc.dram_tensor("x0", [NPC], F32, kind="ExternalInput").ap()
    W1a_d = nc.dram_tensor("W1a", [D, D], dt, kind="ExternalInput").ap()
    w1x_d = nc.dram_tensor("w1x", [1, D], dt, kind="ExternalInput").ap()
    w1t_d = nc.dram_tensor("w1t", [1, D], F32, kind="ExternalInput").ap()
    W2_d = nc.dram_tensor("W2", [D, D], dt, kind="ExternalInput").ap()
    W3_d = nc.dram_tensor("W3", [D, 1], dt, kind="ExternalInput").ap()
    b1_d = nc.dram_tensor("b1", [D, 1], F32, kind="ExternalInput").ap()
    b2_d = nc.dram_tensor("b2", [D, 1], F32, kind="ExternalInput").ap()
    b3_d = nc.dram_tensor("b3", [1, 1], F32, kind="ExternalInput").ap()
    temb_d = nc.dram_tensor("temb", [1, T_STEPS], F32, kind="ExternalInput").ap()
    xout = nc.dram_tensor("xout", [NPC], F32, kind="ExternalOutput").ap()

    a_t, b_t, c_t = _schedule(n_steps)
    ts_list = list(range(T_STEPS - 1, T_STEPS - 1 - n_steps, -1))

    with tile.TileContext(nc) as tc:
        with (
            tc.tile_pool(name="const", bufs=1) as const_pool,
            tc.tile_pool(name="ctx", bufs=1) as ctx_pool,
            tc.tile_pool(name="h1", bufs=4) as h1_pool,
            tc.tile_pool(name="h2", bufs=4) as h2_pool,
            tc.tile_pool(name="pstage", bufs=4) as pstage_pool,
            tc.tile_pool(name="predsq", bufs=2) as predsq_pool,
            tc.tile_pool(name="eps", bufs=2) as eps_pool,
            tc.tile_pool(name="xsq", bufs=2) as xsq_pool,
            tc.tile_pool(name="xrow", bufs=1) as xrow_pool,
            tc.tile_pool(name="xcast", bufs=2) as xcast_pool,
            tc.tile_pool(name="scratch", bufs=2) as scratch_pool,
            tc.tile_pool(name="zp", bufs=3, space="PSUM") as z_pool,
            tc.tile_pool(name="predp", bufs=2, space="PSUM") as pred_pool,
        ):
            # ---------------- ACT table preload ----------------
            # trigger the Silu table-set load (~2.7us) before any real work
            warm = const_pool.tile([D, 1], F32, tag="warm")
            nc.vector.memset(warm[:], 0.0)
            warm2 = const_pool.tile([D, 1], F32, tag="warm2")
            nc.scalar.activation(warm2[:], warm[:],
                                 mybir.ActivationFunctionType.Silu)

            # ---------------- load constants ----------------
            W1a = const_pool.tile([D, D], dt)
            nc.sync.dma_start(W1a[:], W1a_d)
            w1x = const_pool.tile([1, D], dt)
            nc.sync.dma_start(w1x[:], w1x_d)
            W2 = const_pool.tile([D, D], dt)
            nc.sync.dma_start(W2[:], W2_d)
            W3 = const_pool.tile([D, 1], dt)
            nc.sync.dma_start(W3[:], W3_d)
            # padded layer-3 stationaries: col j holds W3, other col 0, so the
            # two 512-col preds of a slot land on adjacent PSUM partitions
            W3c = []
            for j in range(2):
                w = const_pool.tile([D, 2], dt, tag=f"w3c{j}")
                nc.vector.memset(w[:], 0.0)
                nc.vector.tensor_copy(w[:, j:j + 1], W3[:])
                W3c.append(w)
            b1s = const_pool.tile([D, 1], F32)
            nc.sync.dma_start(b1s[:], b1_d)
            b2s = const_pool.tile([D, 1], F32)
            nc.sync.dma_start(b2s[:], b2_d)
            b3s = const_pool.tile([1, 1], F32)
            nc.sync.dma_start(b3s[:], b3_d)
            w1t = const_pool.tile([1, D], F32)
            nc.sync.dma_start(w1t[:], w1t_d)
            temb = const_pool.tile([1, T_STEPS], F32)
            nc.sync.dma_start(temb[:], temb_d)

            # bias_all[d, t] = b1[d] + time_emb[t] * W1[129, d]
            bias_ps = pred_pool.tile([D, T_STEPS], F32, tag="pp", name="bias_ps")
            nc.tensor.matmul(bias_ps[:], w1t[:], temb[:], start=True, stop=True)
            bias_all = const_pool.tile([D, T_STEPS], F32)
            nc.vector.tensor_scalar_add(bias_all[:], bias_ps[:], b1s[:])

            # b3 broadcast to all 128 partitions (for the x-update)
            ones_r = const_pool.tile([1, D], F32)
            nc.vector.memset(ones_r[:], 1.0)
            b3_ps = pred_pool.tile([D, 1], F32, tag="pp", name="b3_ps")
            nc.tensor.matmul(b3_ps[:], ones_r[:], b3s[:], start=True, stop=True)
            b3_bc = const_pool.tile([D, 1], F32)
            nc.vector.tensor_copy(b3_bc[:], b3_ps[:])

            # ---------------- initial x ----------------
            # x square layout per half: [128, 128]; local col n = p*128 + f.
            x_sq = [None, None]
            x_row = [None, None]
            for h in range(2):
                xs = xsq_pool.tile([D, SQF], F32, tag=f"xsq{h}")
                nc.sync.dma_start(
                    xs[:],
                    x0[h * HALF:(h + 1) * HALF].rearrange("(p f) -> p f", p=D),
                )
                x_sq[h] = xs
                xc = xcast_pool.tile([D, SQF], dt, tag=f"xcast{h}")
                nc.vector.tensor_copy(xc[:], xs[:])
                xr = xrow_pool.tile([1, HALF], dt, tag=f"xrow{h}")
                nc.sync.dma_start(xr[:], xc[:])
                x_row[h] = xr

            # ---------------- load context (resident all steps) ----------------
            # split into chunks so the first slots' matmuls can start before
            # the whole 8MB is resident (one big DMA = ~23us serial startup)
            ctx_sb = ctx_pool.tile([D, NPC], dt)
            CCH = NPC // 8
            for ci in range(8):
                nc.sync.dma_start(ctx_sb[:, ci * CCH:(ci + 1) * CCH],
                                  ctxT[:, ci * CCH:(ci + 1) * CCH])

            # ---------------- main loop (flat staggered pipeline) ----------------
            # Global slot k -> (si, h, s). Emission group g issues, per engine
            # FIFO:  PE: z1(g+3), z2(g+2), pred(g)   ACT: silu1(g+2), silu2(g+1)
            # DVE/DMA: drain(g-1).  This keeps every engine queue free of
            # intra-slot waits: ACT runs back-to-back 1024-wide Silus while PE
            # streams the next slots' matmuls.  z-pool rotation (2 allocs/group,
            # 3 bufs) makes z2(k) land exactly in z1(k)'s buffer (in-place).
            # Per-engine emission order is PINNED with nosync deps so the tile
            # scheduler cannot run ahead (its greedy order otherwise batches
            # z1/z2 slots and starves pred/drain, stalling ACT).
            NG = n_steps * 2 * NSLOT

            last_on = {}

            def chain(eng, bi):
                prev = last_on.get(eng)
                if prev is not None:
                    bi.ins.add_dependency(prev.ins.name, DEP_ORDER)
                last_on[eng] = bi
                return bi

            def slot_of(k):
                si, r = divmod(k, 2 * NSLOT)
                h, s = divmod(r, NSLOT)
                return si, ts_list[si], h, s

            # per-half state carried across the pipeline
            state = {h: {} for h in range(2)}

            def emit_prefetch(si, t, h):
                """eps/u prefetch for (si, t, h); called right after x_sq[h]
                for step si is defined (one half-period ahead of use)."""
                st = state[h]
                hoff = h * HALF
                if t > 0:
                    eps = eps_pool.tile([D, SQF], F32, tag=f"eps{h}", name="eps")
                    nc.sync.dma_start(
                        eps[:],
                        noise[si, hoff:hoff + HALF].rearrange("(p f) -> p f", p=D),
                    )
                    eps_s = scratch_pool.tile([D, SQF], F32, tag=f"epss{h}",
                                              name="eps_s")
                    chain("dve", nc.vector.tensor_scalar_mul(
                        eps_s[:], eps[:], float(c_t[t])))
                    st["eps_s"] = eps_s
                else:
                    st["eps_s"] = None
                u = scratch_pool.tile([D, SQF], F32, tag=f"u{h}", name="u")
                chain("dve", nc.vector.tensor_scalar_mul(
                    u[:], x_sq[h][:], float(a_t[t])))
                st["u"] = u

            def front(k):
                si, t, h, s = slot_of(k)
                if s == 0:
                    state[h]["pred_sq"] = predsq_pool.tile(
                        [D, SQF], F32, tag=f"psq{h}", name="pred_sq")
                co = h * HALF + s * SLOT
                lo = s * SLOT
                z1 = z_pool.tile([D, SLOT], F32, tag="z", name="z1")
                for kk in range(2):
                    chain("pe", nc.tensor.matmul(
                        z1[:, 512 * kk:512 * (kk + 1)], W1a[:],
                        ctx_sb[:, co + 512 * kk:co + 512 * (kk + 1)],
                        start=True, stop=False))
                for kk in range(2):
                    chain("pe", nc.tensor.matmul(
                        z1[:, 512 * kk:512 * (kk + 1)], w1x[:],
                        x_row[h][:, lo + 512 * kk:lo + 512 * (kk + 1)],
                        start=False, stop=True))
                return z1

            def mid1(k, z1):
                si, t, h, s = slot_of(k)
                h1 = h1_pool.tile([D, SLOT], dt)
                chain("act", nc.scalar.activation(
                    h1[:], z1[:],
                    mybir.ActivationFunctionType.Silu,
                    bias=bias_all[:, t:t + 1], scale=1.0,
                ))
                return h1

            def mid2(k, h1):
                z2 = z_pool.tile([D, SLOT], F32, tag="z", name="z2")
                for kk in range(2):
                    chain("pe", nc.tensor.matmul(
                        z2[:, 512 * kk:512 * (kk + 1)], W2[:],
                        h1[:, 512 * kk:512 * (kk + 1)],
                        start=True, stop=True))
                return z2

            def mid3(k, z2):
                h2 = h2_pool.tile([D, SLOT], dt)
                chain("act", nc.scalar.activation(
                    h2[:], z2[:],
                    mybir.ActivationFunctionType.Silu,
                    bias=b2s[:], scale=1.0,
                ))
                return h2

            def back(k, h2):
                pp = pred_pool.tile([2, 512], F32, tag="pp", name="pp")
                for j in range(2):
                    chain("pe", nc.tensor.matmul(
                        pp[0:2, 0:512], W3c[j][:],
                        h2[:, 512 * j:512 * (j + 1)],
                        start=(j == 0), stop=(j == 1)))
                return pp

            deferred = {}

            def drain(k, pp):
                si, t, h, s = slot_of(k)
                ps = pstage_pool.tile([2, 512], F32)
                chain("dve", nc.vector.tensor_copy(ps[:], pp[0:2, 0:512]))
                pred_sq = state[h]["pred_sq"]
                nc.sync.dma_start(pred_sq[8 * s:8 * s + 8, :], ps[:])
                if s == NSLOT - 1:
                    # defer the x-update 2 drains so the next half's early
                    # drains are not queued behind it on DVE
                    deferred.setdefault(k + 2, []).append(
                        lambda si=si, t=t, h=h, pred_sq=pred_sq:
                            emit_x_update(si, t, h, pred_sq))
                for fn in deferred.pop(k, []):
                    fn()

            def emit_x_update(si, t, h, pred_sq):
                st = state[h]
                last = si == n_steps - 1
                hoff = h * HALF
                bt = float(b_t[t])
                p = scratch_pool.tile([D, SQF], F32, tag=f"p{h}", name="p")
                chain("dve", nc.vector.tensor_scalar(
                    p[:], pred_sq[:], b3_bc[:], bt,
                    mybir.AluOpType.add, mybir.AluOpType.mult,
                ))
                xs_new = xsq_pool.tile([D, SQF], F32, tag=f"xsq{h}",
                                       name="xs_new")
                if t > 0:
                    v = scratch_pool.tile([D, SQF], F32, tag=f"v{h}", name="v")
                    chain("dve", nc.vector.tensor_tensor(
                        v[:], st["u"][:], p[:], mybir.AluOpType.add))
                    chain("dve", nc.vector.tensor_tensor(
                        xs_new[:], v[:], st["eps_s"][:], mybir.AluOpType.add))
                else:
                    chain("dve", nc.vector.tensor_tensor(
                        xs_new[:], st["u"][:], p[:], mybir.AluOpType.add))
                x_sq[h] = xs_new
                if last:
                    nc.sync.dma_start(
                        xout[hoff:hoff + HALF].rearrange("(p f) -> p f", p=D),
                        xs_new[:],
                    )
                else:
                    xc = xcast_pool.tile([D, SQF], dt, tag=f"xcast{h}")
                    chain("dve", nc.vector.tensor_copy(xc[:], xs_new[:]))
                    xr = xrow_pool.tile([1, HALF], dt, tag=f"xrow{h}")
                    nc.sync.dma_start(xr[:], xc[:])
                    x_row[h] = xr
                    # prefetch for this half's next step (eps, u)
                    emit_prefetch(si + 1, ts_list[si + 1], h)

            # initial prefetches for step 0 (both halves)
            for h in range(2):
                emit_prefetch(0, ts_list[0], h)

            live = {}
            for g in range(-3, NG + 1):
                if 0 <= g + 3 < NG:
                    live[(g + 3, "z1")] = front(g + 3)
                if 0 <= g + 2 < NG:
                    live[(g + 2, "h1")] = mid1(g + 2, live.pop((g + 2, "z1")))
                    live[(g + 2, "z2")] = mid2(g + 2, live.pop((g + 2, "h1")))
                if 0 <= g + 1 < NG:
                    live[(g + 1, "h2")] = mid3(g + 1, live.pop((g + 1, "z2")))
                if 0 <= g < NG:
                    live[(g, "pp")] = back(g, live.pop((g, "h2")))
                if 0 <= g - 1 < NG:
                    drain(g - 1, live.pop((g - 1, "pp")))
            # flush any deferred boundary work past the last drain
            for kk in sorted(deferred):
                for fn in deferred.pop(kk, []):
                    fn()

    nc.compile()
    return nc


_BUILD_CACHE = {}


def _get_nc(n_steps, dt):
    key = (n_steps, str(dt))
    if key not in _BUILD_CACHE:
        _BUILD_CACHE[key] = build(n_steps, dt)
    return _BUILD_CACHE[key]


def _prep_in_maps(context, x_init, noise, W1, b1, W2, b2, W3, b3, time_emb, dt):
    np_dt = np.float32 if dt == F32 else ml_dtypes.bfloat16
    in_maps = []
    W1a = np.ascontiguousarray(W1[:D].astype(np_dt))
    w1x = np.ascontiguousarray(W1[D:D + 1].astype(np_dt))
    w1t = np.ascontiguousarray(W1[D + 1:D + 2].astype(np.float32))
    W2c = np.ascontiguousarray(W2.astype(np_dt))
    W3c = np.ascontiguousarray(W3.astype(np_dt))
    b1c = np.ascontiguousarray(b1.reshape(D, 1).astype(np.float32))
    b2c = np.ascontiguousarray(b2.reshape(D, 1).astype(np.float32))
    b3c = np.ascontiguousarray(b3.reshape(1, 1).astype(np.float32))
    tec = np.ascontiguousarray(time_emb.reshape(1, T_STEPS).astype(np.float32))
    for c in range(N_CORES):
        s = slice(c * NPC, (c + 1) * NPC)
        in_maps.append({
            "ctxT": np.ascontiguousarray(context[s].T.astype(np_dt)),
            "noise": np.ascontiguousarray(noise[:, s, 0].astype(np.float32)),
            "x0": np.ascontiguousarray(x_init[s, 0].astype(np.float32)),
            "W1a": W1a, "w1x": w1x, "w1t": w1t,
            "W2": W2c, "W3": W3c,
            "b1": b1c, "b2": b2c, "b3": b3c,
            "temb": tec,
        })
    return in_maps


def run(inputs, n_steps=T_STEPS, dt=None, trace=False, tmpdir=None):
    if dt is None:
        dt = F32 if os.environ.get("K_DT", "bf16") == "f32" else BF16
    nc = _get_nc(n_steps, dt)
    in_maps = _prep_in_maps(**{k: np.asarray(v) for k, v in inputs.items()}, dt=dt)
    res = bass_utils.run_bass_kernel_spmd(
        nc, in_maps, list(range(N_CORES)), trace=trace, tmpdir=tmpdir,
    )
    out = np.concatenate([res.results[c]["xout"] for c in range(N_CORES)])
    return out.reshape(B, 1).astype(np.float32), res


def kernel(**inputs):
    out, _ = run(inputs)
    return out


# revision 6
# speedup vs baseline: 1.0000x; 1.0000x over previous
"""Trainium2 Bass kernel for nn_DiffusionHead: 100-step diffusion sampling of a
tiny MLP head (130->128->128->1) over a batch of 262144 rows.

v2 design ("3-set PSUM rotation", ACT-bound ~6.6ms target):
  - Pure data parallel over 8 NeuronCores, 32768 rows/core, batch split in two
    halves of 16384 that alternate at STEP granularity (the x-recurrence tail
    of one half hides under the other half's 8 compute slots).
  - Layout: features d (128) on partitions, batch n on the free dim.
  - Slot = 1024 batch cols. Per slot: z1 = W1a@ctx + w1x@x_row accumulated in
    one rotating PSUM tile [128,1024] (2 banks); ONE 1024-wide Silu (bias =
    b1 + time_emb[t]*W1[129] folded per-partition) -> h1 bf16; z2 = W2@h1 into
    the next rotating PSUM tile; ONE 1024-wide Silu -> h2 bf16; pred = W3c@h2
    (2 x 512-col matmuls, zero-padded stationary trick) into a dedicated
    double-buffered 1-bank pred pool; DVE drains pred [2,512] -> SBUF; DMA
    scatters it into the square pred_sq [128,128] layout.
  - PSUM budget: 3 z-tiles x 2 banks + 2 pred bufs x 1 bank = 8 banks exactly.
  - ACT does exactly 2 x (1024+222)cyc instructions per slot => ~66.4us/step;
    PE does 4096 cyc/slot (~82% duty, stays HAM-warm at 2.4GHz).
  - x-update runs on DVE in the square [128,128] layout with schedule constants
    folded as immediates; x_row (bf16 row layout for the rank-1 matmul) is
    rebuilt via cast + SBUF->SBUF DMA once per half-step.
"""

import os
import numpy as np
import ml_dtypes

import bass_rust
import concourse.bass as bass
import concourse.bacc as bacc
import concourse.mybir as mybir
from concourse import tile
from concourse import bass_utils

DEP_ORDER = bass_rust.DependencyInfo(sync=False, no_sync=True)

B = 262144
D = 128
T_STEPS = 100
N_CORES = 8
NPC = B // N_CORES          # 32768 rows per core
HALF = NPC // 2             # 16384
SLOT = 1024                 # batch cols per slot
NSLOT = HALF // SLOT        # 8 slots per half
SQF = HALF // D             # 128 free cols of the square layout
BETA_START = 1e-4
BETA_END = 0.02

F32 = mybir.dt.float32
BF16 = mybir.dt.bfloat16


def _schedule(n_steps):
    """Compile-time diffusion schedule constants (pure linspace math; no input
    data involved). Computed in float64 for accuracy."""
    betas = np.linspace(BETA_START, BETA_END, T_STEPS, dtype=np.float64)
    alphas = 1.0 - betas
    acp = np.cumprod(alphas)
    a_t = 1.0 / np.sqrt(alphas)                       # x coefficient
    b_t = -betas / (np.sqrt(1.0 - acp) * np.sqrt(alphas))  # pred coefficient
    c_t = np.sqrt(betas)                              # eps coefficient
    return a_t, b_t, c_t


def build(n_steps=T_STEPS, dt=BF16):
    nc = bacc.Bacc("TRN2", target_bir_lowering=False, debug=False)

    # ---------------- DRAM tensors (per-core inputs) ----------------
    ctxT = nc.dram_tensor("ctxT", [D, NPC], dt, kind="ExternalInput").ap()
    noise = nc.dram_tensor("noise", [T_STEPS, NPC], F32, kind="ExternalInput").ap()
    x0 = nc.dram_tensor("x0", [NPC], F32, kind="ExternalInput").ap()
    W1a_d = nc.dram_tensor("W1a", [D, D], dt, kind="ExternalInput").ap()
    w1x_d = nc.dram_tensor("w1x", [1, D], dt, kind="ExternalInput").ap()
    w1t_d = nc.dram_tensor("w1t", [1, D], F32, kind="ExternalInput").ap()
    W2_d = nc.dram_tensor("W2", [D, D], dt, kind="ExternalInput").ap()
    W3_d = nc.dram_tensor("W3", [D, 1], dt, kind="ExternalInput").ap()
    b1_d = nc.dram_tensor("b1", [D, 1], F32, kind="ExternalInput").ap()
    b2_d = nc.dram_tensor("b2", [D, 1], F32, kind="ExternalInput").ap()
    b3_d = nc.dram_tensor("b3", [1, 1], F32, kind="ExternalInput").ap()
    temb_d = nc.dram_tensor("temb", [1, T_STEPS], F32, kind="ExternalInput").ap()
    xout = nc.dram_tensor("xout", [NPC], F32, kind="ExternalOutput").ap()

    a_t, b_t, c_t = _schedule(n_steps)
    ts_list = list(range(T_STEPS - 1, T_STEPS - 1 - n_steps, -1))

    with tile.TileContext(nc) as tc:
        with (
            tc.tile_pool(name="const", bufs=1) as const_pool,
            tc.tile_pool(name="ctx", bufs=1) as ctx_pool,
            tc.tile_pool(name="h1", bufs=3) as h1_pool,
            tc.tile_pool(name="h2", bufs=3) as h2_pool,
            tc.tile_pool(name="pstage", bufs=3) as pstage_pool,
            tc.tile_pool(name="predsq", bufs=2) as predsq_pool,
            tc.tile_pool(name="eps", bufs=2) as eps_pool,
            tc.tile_pool(name="xsq", bufs=2) as xsq_pool,
            tc.tile_pool(name="xrow", bufs=1) as xrow_pool,
            tc.tile_pool(name="xcast", bufs=2) as xcast_pool,
            tc.tile_pool(name="scratch", bufs=2) as scratch_pool,
            tc.tile_pool(name="zp", bufs=3, space="PSUM") as z_pool,
            tc.tile_pool(name="predp", bufs=2, space="PSUM") as pred_pool,
        ):
            # ---------------- load constants ----------------
            W1a = const_pool.tile([D, D], dt)
            nc.sync.dma_start(W1a[:], W1a_d)
            w1x = const_pool.tile([1, D], dt)
            nc.sync.dma_start(w1x[:], w1x_d)
            W2 = const_pool.tile([D, D], dt)
            nc.sync.dma_start(W2[:], W2_d)
            W3 = const_pool.tile([D, 1], dt)
            nc.sync.dma_start(W3[:], W3_d)
            # padded layer-3 stationaries: col j holds W3, other col 0, so the
            # two 512-col preds of a slot land on adjacent PSUM partitions
            W3c = []
            for j in range(2):
                w = const_pool.tile([D, 2], dt, tag=f"w3c{j}")
                nc.vector.memset(w[:], 0.0)
                nc.vector.tensor_copy(w[:, j:j + 1], W3[:])
                W3c.append(w)
            b1s = const_pool.tile([D, 1], F32)
            nc.sync.dma_start(b1s[:], b1_d)
            b2s = const_pool.tile([D, 1], F32)
            nc.sync.dma_start(b2s[:], b2_d)
            b3s = const_pool.tile([1, 1], F32)
            nc.sync.dma_start(b3s[:], b3_d)
            w1t = const_pool.tile([1, D], F32)
            nc.sync.dma_start(w1t[:], w1t_d)
            temb = const_pool.tile([1, T_STEPS], F32)
            nc.sync.dma_start(temb[:], temb_d)

            # bias_all[d, t] = b1[d] + time_emb[t] * W1[129, d]
            bias_ps = pred_pool.tile([D, T_STEPS], F32, tag="pp", name="bias_ps")
            nc.tensor.matmul(bias_ps[:], w1t[:], temb[:], start=True, stop=True)
            bias_all = const_pool.tile([D, T_STEPS], F32)
            nc.vector.tensor_scalar_add(bias_all[:], bias_ps[:], b1s[:])

            # b3 broadcast to all 128 partitions (for the x-update)
            ones_r = const_pool.tile([1, D], F32)
            nc.vector.memset(ones_r[:], 1.0)
            b3_ps = pred_pool.tile([D, 1], F32, tag="pp", name="b3_ps")
            nc.tensor.matmul(b3_ps[:], ones_r[:], b3s[:], start=True, stop=True)
            b3_bc = const_pool.tile([D, 1], F32)
            nc.vector.tensor_copy(b3_bc[:], b3_ps[:])

            # ---------------- initial x ----------------
            # x square layout per half: [128, 128]; local col n = p*128 + f.
            x_sq = [None, None]
            x_row = [None, None]
            for h in range(2):
                xs = xsq_pool.tile([D, SQF], F32, tag=f"xsq{h}")
                nc.sync.dma_start(
                    xs[:],
                    x0[h * HALF:(h + 1) * HALF].rearrange("(p f) -> p f", p=D),
                )
                x_sq[h] = xs
                xc = xcast_pool.tile([D, SQF], dt, tag=f"xcast{h}")
                nc.vector.tensor_copy(xc[:], xs[:])
                xr = xrow_pool.tile([1, HALF], dt, tag=f"xrow{h}")
                nc.sync.dma_start(xr[:], xc[:])
                x_row[h] = xr

            # ---------------- load context (resident all steps) ----------------
            # split into chunks so the first slots' matmuls can start before
            # the whole 8MB is resident (one big DMA = ~23us serial startup)
            ctx_sb = ctx_pool.tile([D, NPC], dt)
            CCH = NPC // 8
            for ci in range(8):
                nc.sync.dma_start(ctx_sb[:, ci * CCH:(ci + 1) * CCH],
                                  ctxT[:, ci * CCH:(ci + 1) * CCH])

            # ---------------- main loop (flat staggered pipeline) ----------------
            # Global slot k -> (si, h, s). Emission group g issues, per engine
            # FIFO:  PE: z1(g+3), z2(g+2), pred(g)   ACT: silu1(g+2), silu2(g+1)
            # DVE/DMA: drain(g-1).  This keeps every engine queue free of
            # intra-slot waits: ACT runs back-to-back 1024-wide Silus while PE
            # streams the next slots' matmuls.  z-pool rotation (2 allocs/group,
            # 3 bufs) makes z2(k) land exactly in z1(k)'s buffer (in-place).
            # Per-engine emission order is PINNED with nosync deps so the tile
            # scheduler cannot run ahead (its greedy order otherwise batches
            # z1/z2 slots and starves pred/drain, stalling ACT).
            NG = n_steps * 2 * NSLOT

            last_on = {}

            def chain(eng, bi):
                prev = last_on.get(eng)
                if prev is not None:
                    bi.ins.add_dependency(prev.ins.name, DEP_ORDER)
                last_on[eng] = bi
                return bi

            def slot_of(k):
                si, r = divmod(k, 2 * NSLOT)
                h, s = divmod(r, NSLOT)
                return si, ts_list[si], h, s

            # per-half state carried across the pipeline
            state = {h: {} for h in range(2)}

            def emit_prefetch(si, t, h):
                """eps/u prefetch for (si, t, h); called right after x_sq[h]
                for step si is defined (one half-period ahead of use)."""
                st = state[h]
                hoff = h * HALF
                if t > 0:
                    eps = eps_pool.tile([D, SQF], F32, tag=f"eps{h}", name="eps")
                    nc.sync.dma_start(
                        eps[:],
                        noise[si, hoff:hoff + HALF].rearrange("(p f) -> p f", p=D),
                    )
                    eps_s = scratch_pool.tile([D, SQF], F32, tag=f"epss{h}",
                                              name="eps_s")
                    chain("dve", nc.vector.tensor_scalar_mul(
                        eps_s[:], eps[:], float(c_t[t])))
                    st["eps_s"] = eps_s
                else:
                    st["eps_s"] = None
                u = scratch_pool.tile([D, SQF], F32, tag=f"u{h}", name="u")
                chain("dve", nc.vector.tensor_scalar_mul(
                    u[:], x_sq[h][:], float(a_t[t])))
                st["u"] = u

            def front(k):
                si, t, h, s = slot_of(k)
                if s == 0:
                    state[h]["pred_sq"] = predsq_pool.tile(
                        [D, SQF], F32, tag=f"psq{h}", name="pred_sq")
                co = h * HALF + s * SLOT
                lo = s * SLOT
                z1 = z_pool.tile([D, SLOT], F32, tag="z", name="z1")
                for kk in range(2):
                    chain("pe", nc.tensor.matmul(
                        z1[:, 512 * kk:512 * (kk + 1)], W1a[:],
                        ctx_sb[:, co + 512 * kk:co + 512 * (kk + 1)],
                        start=True, stop=False))
                for kk in range(2):
                    chain("pe", nc.tensor.matmul(
                        z1[:, 512 * kk:512 * (kk + 1)], w1x[:],
                        x_row[h][:, lo + 512 * kk:lo + 512 * (kk + 1)],
                        start=False, stop=True))
                return z1

            def mid1(k, z1):
                si, t, h, s = slot_of(k)
                h1 = h1_pool.tile([D, SLOT], dt)
                chain("act", nc.scalar.activation(
                    h1[:], z1[:],
                    mybir.ActivationFunctionType.Silu,
                    bias=bias_all[:, t:t + 1], scale=1.0,
                ))
                return h1

            def mid2(k, h1):
                z2 = z_pool.tile([D, SLOT], F32, tag="z", name="z2")
                for kk in range(2):
                    chain("pe", nc.tensor.matmul(
                        z2[:, 512 * kk:512 * (kk + 1)], W2[:],
                        h1[:, 512 * kk:512 * (kk + 1)],
                        start=True, stop=True))
                return z2

            def mid3(k, z2):
                h2 = h2_pool.tile([D, SLOT], dt)
                chain("act", nc.scalar.activation(
                    h2[:], z2[:],
                    mybir.ActivationFunctionType.Silu,
                    bias=b2s[:], scale=1.0,
                ))
                return h2

            def back(k, h2):
                pp = pred_pool.tile([2, 512], F32, tag="pp", name="pp")
                for j in range(2):
                    chain("pe", nc.tensor.matmul(
                        pp[0:2, 0:512], W3c[j][:],
                        h2[:, 512 * j:512 * (j + 1)],
                        start=(j == 0), stop=(j == 1)))
                return pp

            deferred = {}

            def drain(k, pp):
                si, t, h, s = slot_of(k)
                ps = pstage_pool.tile([2, 512], F32)
                chain("dve", nc.vector.tensor_copy(ps[:], pp[0:2, 0:512]))
                pred_sq = state[h]["pred_sq"]
                nc.sync.dma_start(pred_sq[8 * s:8 * s + 8, :], ps[:])
                if s == NSLOT - 1:
                    # defer the x-update 2 drains so the next half's early
                    # drains are not queued behind it on DVE
                    deferred.setdefault(k + 2, []).append(
                        lambda si=si, t=t, h=h, pred_sq=pred_sq:
                            emit_x_update(si, t, h, pred_sq))
                for fn in deferred.pop(k, []):
                    fn()

            def emit_x_update(si, t, h, pred_sq):
                st = state[h]
                last = si == n_steps - 1
                hoff = h * HALF
                bt = float(b_t[t])
                p = scratch_pool.tile([D, SQF], F32, tag=f"p{h}", name="p")
                chain("dve", nc.vector.tensor_scalar(
                    p[:], pred_sq[:], b3_bc[:], bt,
                    mybir.AluOpType.add, mybir.AluOpType.mult,
                ))
                xs_new = xsq_pool.tile([D, SQF], F32, tag=f"xsq{h}",
                                       name="xs_new")
                if t > 0:
                    v = scratch_pool.tile([D, SQF], F32, tag=f"v{h}", name="v")
                    chain("dve", nc.vector.tensor_tensor(
                        v[:], st["u"][:], p[:], mybir.AluOpType.add))
                    chain("dve", nc.vector.tensor_tensor(
                        xs_new[:], v[:], st["eps_s"][:], mybir.AluOpType.add))
                else:
                    chain("dve", nc.vector.tensor_tensor(
                        xs_new[:], st["u"][:], p[:], mybir.AluOpType.add))
                x_sq[h] = xs_new
                if last:
                    nc.sync.dma_start(
                        xout[hoff:hoff + HALF].rearrange("(p f) -> p f", p=D),
                        xs_new[:],
                    )
                else:
                    xc = xcast_pool.tile([D, SQF], dt, tag=f"xcast{h}")
                    chain("dve", nc.vector.tensor_copy(xc[:], xs_new[:]))
                    xr = xrow_pool.tile([1, HALF], dt, tag=f"xrow{h}")
                    nc.sync.dma_start(xr[:], xc[:])
                    x_row[h] = xr
                    # prefetch for this half's next step (eps, u)
                    emit_prefetch(si + 1, ts_list[si + 1], h)

            # initial prefetches for step 0 (both halves)
            for h in range(2):
                emit_prefetch(0, ts_list[0], h)

            live = {}
            for g in range(-3, NG + 1):
                if 0 <= g + 3 < NG:
                    live[(g + 3, "z1")] = front(g + 3)
                if 0 <= g + 2 < NG:
                    live[(g + 2, "h1")] = mid1(g + 2, live.pop((g + 2, "z1")))
                    live[(g + 2, "z2")] = mid2(g + 2, live.pop((g + 2, "h1")))
                if 0 <= g + 1 < NG:
                    live[(g + 1, "h2")] = mid3(g + 1, live.pop((g + 1, "z2")))
                if 0 <= g < NG:
                    live[(g, "pp")] = back(g, live.pop((g, "h2")))
                if 0 <= g - 1 < NG:
                    drain(g - 1, live.pop((g - 1, "pp")))
            # flush any deferred boundary work past the last drain
            for kk in sorted(deferred):
                for fn in deferred.pop(kk, []):
                    fn()

    nc.compile()
    return nc


_BUILD_CACHE = {}


def _get_nc(n_steps, dt):
    key = (n_steps, str(dt))
    if key not in _BUILD_CACHE:
        _BUILD_CACHE[key] = build(n_steps, dt)
    return _BUILD_CACHE[key]


def _prep_in_maps(context, x_init, noise, W1, b1, W2, b2, W3, b3, time_emb, dt):
    np_dt = np.float32 if dt == F32 else ml_dtypes.bfloat16
    in_maps = []
    W1a = np.ascontiguousarray(W1[:D].astype(np_dt))
    w1x = np.ascontiguousarray(W1[D:D + 1].astype(np_dt))
    w1t = np.ascontiguousarray(W1[D + 1:D + 2].astype(np.float32))
    W2c = np.ascontiguousarray(W2.astype(np_dt))
    W3c = np.ascontiguousarray(W3.astype(np_dt))
    b1c = np.ascontiguousarray(b1.reshape(D, 1).astype(np.float32))
    b2c = np.ascontiguousarray(b2.reshape(D, 1).astype(np.float32))
    b3c = np.ascontiguousarray(b3.reshape(1, 1).astype(np.float32))
    tec = np.ascontiguousarray(time_emb.reshape(1, T_STEPS).astype(np.float32))
    for c in range(N_CORES):
        s = slice(c * NPC, (c + 1) * NPC)
        in_maps.append({
            "ctxT": np.ascontiguousarray(context[s].T.astype(np_dt)),
            "noise": np.ascontiguousarray(noise[:, s, 0].astype(np.float32)),
            "x0": np.ascontiguousarray(x_init[s, 0].astype(np.float32)),
            "W1a": W1a, "w1x": w1x, "w1t": w1t,
            "W2": W2c, "W3": W3c,
            "b1": b1c, "b2": b2c, "b3": b3c,
            "temb": tec,
        })
    return in_maps


def run(inputs, n_steps=T_STEPS, dt=None, trace=False, tmpdir=None):
    if dt is None:
        dt = F32 if os.environ.get("K_DT", "bf16") == "f32" else BF16
    nc = _get_nc(n_steps, dt)
    in_maps = _prep_in_maps(**{k: np.asarray(v) for k, v in inputs.items()}, dt=dt)
    res = bass_utils.run_bass_kernel_spmd(
        nc, in_maps, list(range(N_CORES)), trace=trace, tmpdir=tmpdir,
    )
    out = np.concatenate([res.results[c]["xout"] for c in range(N_CORES)])
    return out.reshape(B, 1).astype(np.float32), res


def kernel(**inputs):
    out, _ = run(inputs)
    return out
